# revision 17
# baseline (speedup 1.0000x reference)
"""Butterfly sparse-attention MLP kernel for 8 Trainium2 NeuronCores.

Computation (from the reference):
    attn = (w1.T @ w2.T) * sparse_mask          # [4096 s, 4096 t]
    y    = gelu(x @ attn + b2)                  # [8, 768, 4096]

sparse_mask is banded: mask[s, t] == 0 whenever |s - t| > 133.  Each core
owns a 512-wide t-block and only needs an 896-wide s-window around it.
Per t-subtile of 128, only 4 of the 7 s-chunks in the window can carry
non-zero attn, so phase B contracts over 512 of s instead of 4096, and
phase A only computes the in-band t-columns of each attn chunk.

Sharding: tensor-parallel over t (8 blocks of 512).  All per-core variation
is in the input data (windows are zero-padded at the edges; mask zeros make
padded contributions exactly zero), so one SPMD BIR serves all 8 cores.

Matmul operands travel as fp16 (10-bit mantissa; values here are O(1), and
accumulation stays fp32 in PSUM) which halves HBM traffic.  Weight tensors
are host-shuffled so each DMA descriptor is 3.5-4 KB — the HW-DGE queues
are descriptor-rate limited (~60 M/s), not byte limited.  Streams are
spread over the sync/scalar HW-DGE queues plus the gpsimd SW-DGE queue.
"""

import numpy as np

B, T, D = 8, 768, 4096
N = B * T            # 6144 rows of x
NCORES = 8
TB = 512             # t-columns per core
P = 128
MARGIN = 192         # s-window extends this far before/after the t-block
SW = TB + 2 * MARGIN  # 896 s-window width
NCH = SW // P        # 7 s-chunks
DCH = D // P         # 32 d-chunks (contraction of phase A)
NQ = TB // P         # 4 t-subtiles per core
GN = 2048            # n-group width in phase B
NG = N // GN         # 3 n-groups
MMN = 512            # moving-operand / PSUM-bank free-dim cap per matmul
BANDCH = 4           # s-chunks feeding one t-subtile (covers +-133 band)
W1PACK = 2           # w1 d-chunks packed per DMA row (3.5 KB descriptors)
W2PACK = 4           # w2T d-chunks packed per DMA row (4 KB descriptors)

_NC = None


def _band(j):
    """t-column range [lo, hi) of attn chunk j that phase B reads."""
    lo = P * max(0, j - (BANDCH - 1))
    hi = P * min(NQ - 1, j) + P
    return lo, hi


def _build_module():
    from concourse import bacc, bass, mybir, tile

    f32 = mybir.dt.float32
    f16 = mybir.dt.float16
    PSUM = bass.MemorySpace.PSUM

    nc = bacc.Bacc("TRN2", target_bir_lowering=False, debug=False)
    xT_d = nc.declare_dram_parameter("xT_s", [NCH, P, N], f16, isOutput=False)
    w1_d = nc.declare_dram_parameter(
        "w1_s", [DCH // W1PACK, P, W1PACK * SW], f16, isOutput=False)
    w2T_d = nc.declare_dram_parameter(
        "w2T_s", [DCH // W2PACK, P, W2PACK * TB], f16, isOutput=False)
    mask_d = nc.declare_dram_parameter("mask_s", [SW, TB], f16, isOutput=False)
    b2_d = nc.declare_dram_parameter("b2c_s", [P, NQ], f32, isOutput=False)
    yT_d = nc.declare_dram_parameter("yT_s", [TB, N], f16, isOutput=True)

    with tile.TileContext(nc) as tc:
        with (
            tc.tile_pool(name="const", bufs=1) as cpool,
            tc.tile_pool(name="attn", bufs=1) as apool,
            tc.tile_pool(name="mp", bufs=1) as mp,
            tc.tile_pool(name="xp", bufs=2 * NCH) as xp,
            tc.tile_pool(name="yp", bufs=3) as yp,
        ):
            b2_t = cpool.tile([P, NQ], f32)
            nc.sync.dma_start(b2_t[:], b2_d[:])

            # Masks land early via the (otherwise idle) SW-DGE queue.
            m_ts = []
            for j in range(NCH):
                m_t = mp.tile([P, TB], f16, name=f"m_t{j}")
                nc.gpsimd.dma_start(m_t[:], mask_d[j * P:(j + 1) * P, :])
                m_ts.append(m_t)

            engs = [nc.sync, nc.scalar, nc.gpsimd]

            # ---- Phase A: attn[s, t] = (w1.T @ w2T) * mask on the band ----
            attn_sb = []
            with (
                tc.tile_pool(name="w1p", bufs=3) as w1p,
                tc.tile_pool(name="w2p", bufs=3) as w2p,
                tc.tile_pool(name="psA", bufs=1, space=PSUM) as psA,
            ):
                attn_ps = [
                    psA.tile([P, TB], f32, name=f"attn_ps{j}") for j in range(NCH)
                ]
                ndma = 0
                for bb in range(DCH // W2PACK):
                    w2_t = w2p.tile([P, W2PACK * TB], f16)
                    engs[ndma % 3].dma_start(w2_t[:], w2T_d[bb])
                    ndma += 1
                    for hb in range(W2PACK // W1PACK):
                        w1_t = w1p.tile([P, W1PACK * SW], f16)
                        engs[ndma % 3].dma_start(
                            w1_t[:], w1_d[bb * (W2PACK // W1PACK) + hb])
                        ndma += 1
                        for half in range(W1PACK):
                            k = bb * W2PACK + hb * W1PACK + half
                            w1sl = w1_t[:, half * SW:(half + 1) * SW]
                            w2sl = w2_t[:, (hb * W1PACK + half) * TB:
                                        (hb * W1PACK + half + 1) * TB]
                            for j in range(NCH):
                                lo, hi = _band(j)
                                nc.tensor.matmul(
                                    attn_ps[j][:, lo:hi],
                                    w1sl[:, j * P:(j + 1) * P],
                                    w2sl[:, lo:hi],
                                    start=(k == 0),
                                    stop=(k == DCH - 1),
                                )
                for j in range(NCH):
                    lo, hi = _band(j)
                    a_t = apool.tile([P, TB], f16, name=f"attn_sb{j}")
                    nc.vector.tensor_mul(
                        a_t[:, lo:hi], attn_ps[j][:, lo:hi], m_ts[j][:, lo:hi]
                    )
                    attn_sb.append(a_t)

            # ---- Phase B: yT[t, n] = gelu(attn.T @ xT + b2) on the band ----
            with tc.tile_pool(name="psB", bufs=2, space=PSUM) as psB:
                xdma = 0
                for g in range(NG):
                    x_t = []
                    for j in range(NCH):
                        xt = xp.tile([P, GN], f16, name="x_t", tag="x_t")
                        engs[xdma % 3].dma_start(
                            xt[:], xT_d[j, :, g * GN:(g + 1) * GN]
                        )
                        xdma += 1
                        x_t.append(xt)
                    for q in range(NQ):
                        y_ps = psB.tile([P, GN], f32)
                        for h in range(GN // MMN):
                            nsl = slice(h * MMN, (h + 1) * MMN)
                            for c in range(BANDCH):
                                j = q + c
                                nc.tensor.matmul(
                                    y_ps[:, nsl],
                                    attn_sb[j][:, q * P:(q + 1) * P],
                                    x_t[j][:, nsl],
                                    start=(c == 0),
                                    stop=(c == BANDCH - 1),
                                )
                        y_sb = yp.tile([P, GN], f16)
                        nc.scalar.activation(
                            y_sb[:],
                            y_ps[:],
                            mybir.ActivationFunctionType.Gelu,
                            bias=b2_t[:, q:q + 1],
                            scale=1.0,
                        )
                        st_eng = nc.sync if q % 2 == 0 else nc.scalar
                        st_eng.dma_start(
                            yT_d[q * P:(q + 1) * P, g * GN:(g + 1) * GN], y_sb[:]
                        )

    nc.compile()
    nc.finalize()
    return nc


def _get_nc():
    global _NC
    if _NC is None:
        _NC = _build_module()
    return _NC


def prepare_in_maps(x, w1, w2, b2, sparse_mask):
    x = np.asarray(x, dtype=np.float32)
    w1 = np.asarray(w1, dtype=np.float32)
    w2 = np.asarray(w2, dtype=np.float32)
    b2 = np.asarray(b2, dtype=np.float32)
    sparse_mask = np.asarray(sparse_mask, dtype=np.float32)

    xT = np.ascontiguousarray(x.reshape(N, D).T.astype(np.float16))   # [s, n]
    w2T = np.ascontiguousarray(w2.T.astype(np.float16))               # [d, t]

    # Zero-pad the s axis by MARGIN on both sides so every core's window is
    # a plain slice; mask zeros make the padded rows contribute nothing.
    xT_pad = np.zeros((D + 2 * MARGIN, N), dtype=np.float16)
    xT_pad[MARGIN:MARGIN + D] = xT
    w1_pad = np.zeros((D, D + 2 * MARGIN), dtype=np.float16)
    w1_pad[:, MARGIN:MARGIN + D] = w1.astype(np.float16)
    mask_pad = np.zeros((D + 2 * MARGIN, D), dtype=np.float16)
    mask_pad[MARGIN:MARGIN + D] = sparse_mask.astype(np.float16)

    in_maps = []
    for i in range(NCORES):
        s0 = i * TB           # window start in padded coords
        t0 = i * TB
        w1win = w1_pad[:, s0:s0 + SW]                     # [D, SW]
        # pack W1PACK d-chunks per DMA row: [DCH/W1PACK, P, W1PACK*SW]
        w1_s = (w1win.reshape(DCH // W1PACK, W1PACK, P, SW)
                .transpose(0, 2, 1, 3)
                .reshape(DCH // W1PACK, P, W1PACK * SW))
        w2win = w2T[:, t0:t0 + TB]                        # [D, TB]
        w2_s = (w2win.reshape(DCH // W2PACK, W2PACK, P, TB)
                .transpose(0, 2, 1, 3)
                .reshape(DCH // W2PACK, P, W2PACK * TB))
        in_maps.append({
            "xT_s": np.ascontiguousarray(
                xT_pad[s0:s0 + SW].reshape(NCH, P, N)),
            "w1_s": np.ascontiguousarray(w1_s),
            "w2T_s": np.ascontiguousarray(w2_s),
            "mask_s": np.ascontiguousarray(mask_pad[s0:s0 + SW, t0:t0 + TB]),
            "b2c_s": np.ascontiguousarray(b2[t0:t0 + TB].reshape(NQ, P).T),
        })
    return in_maps


def assemble(results):
    out = np.empty((N, D), dtype=np.float32)
    for i in range(NCORES):
        out[:, i * TB:(i + 1) * TB] = results[i]["yT_s"].T.astype(np.float32)
    return out.reshape(B, T, D)


def kernel(x, w1, w2, b2, sparse_mask):
    from concourse.bass_utils import run_bass_kernel_spmd

    in_maps = prepare_in_maps(x, w1, w2, b2, sparse_mask)
    nc = _get_nc()
    res = run_bass_kernel_spmd(nc, in_maps, list(range(NCORES)))
    return assemble(res.results)


# revision 18
# speedup vs baseline: 1.1547x; 1.1547x over previous
"""Butterfly sparse-attention MLP kernel for 8 Trainium2 NeuronCores.

Computation (from the reference):
    attn = (w1.T @ w2.T) * sparse_mask          # [4096 s, 4096 t]
    y    = gelu(x @ attn + b2)                  # [8, 768, 4096]

sparse_mask is banded: mask[s, t] == 0 whenever |s - t| > 133.  Each core
owns a 512-wide t-block and only needs an 896-wide s-window around it.
Per t-subtile of 128, only 4 of the 7 s-chunks in the window can carry
non-zero attn, so phase B contracts over 512 of s instead of 4096, and
phase A only computes the in-band t-columns of each attn chunk.

Sharding: tensor-parallel over t (8 blocks of 512).  All per-core variation
is in the input data (windows are zero-padded at the edges; mask zeros make
padded contributions exactly zero), so one SPMD BIR serves all 8 cores.

Matmul operands travel as fp16 (10-bit mantissa; values here are O(1), and
accumulation stays fp32 in PSUM) which halves HBM traffic.  Weight tensors
are host-shuffled so each DMA descriptor is 3.5-4 KB — the HW-DGE queues
are descriptor-rate limited (~60 M/s), not byte limited.  Streams are
spread over the sync/scalar HW-DGE queues plus the gpsimd SW-DGE queue.
"""

import numpy as np

B, T, D = 8, 768, 4096
N = B * T            # 6144 rows of x
NCORES = 8
TB = 512             # t-columns per core
P = 128
MARGIN = 192         # s-window extends this far before/after the t-block
SW = TB + 2 * MARGIN  # 896 s-window width
NCH = SW // P        # 7 s-chunks
DCH = D // P         # 32 d-chunks (contraction of phase A)
NQ = TB // P         # 4 t-subtiles per core
GN = 2048            # n-group width in phase B
NG = N // GN         # 3 n-groups
MMN = 512            # moving-operand / PSUM-bank free-dim cap per matmul
BANDCH = 4           # s-chunks feeding one t-subtile (covers +-133 band)
W1PACK = 2           # w1 d-chunks packed per DMA row (3.5 KB descriptors)
W2PACK = 4           # w2T d-chunks packed per DMA row (4 KB descriptors)

_NC = None


def _band(j):
    """t-column range [lo, hi) of attn chunk j that phase B reads."""
    lo = P * max(0, j - (BANDCH - 1))
    hi = P * min(NQ - 1, j) + P
    return lo, hi


def _build_module():
    from concourse import bacc, bass, mybir, tile

    f32 = mybir.dt.float32
    f16 = mybir.dt.float16
    PSUM = bass.MemorySpace.PSUM

    nc = bacc.Bacc("TRN2", target_bir_lowering=False, debug=False)
    xT_d = nc.declare_dram_parameter("xT_s", [NCH, P, N], f16, isOutput=False)
    w1_d = nc.declare_dram_parameter(
        "w1_s", [DCH // W1PACK, P, W1PACK * SW], f16, isOutput=False)
    w2T_d = nc.declare_dram_parameter(
        "w2T_s", [DCH // W2PACK, P, W2PACK * TB], f16, isOutput=False)
    mask_d = nc.declare_dram_parameter("mask_s", [SW, TB], f16, isOutput=False)
    b2_d = nc.declare_dram_parameter("b2c_s", [P, NQ], f32, isOutput=False)
    yT_d = nc.declare_dram_parameter("yT_s", [TB, N], f16, isOutput=True)

    with tile.TileContext(nc) as tc:
        with (
            tc.tile_pool(name="const", bufs=1) as cpool,
            tc.tile_pool(name="attn", bufs=1) as apool,
            tc.tile_pool(name="mp", bufs=1) as mp,
            tc.tile_pool(name="xp", bufs=NG * NCH) as xp,
            tc.tile_pool(name="yp", bufs=3) as yp,
        ):
            b2_t = cpool.tile([P, NQ], f32)
            nc.sync.dma_start(b2_t[:], b2_d[:])

            # Masks land early via the (otherwise idle) SW-DGE queue.
            m_ts = []
            for j in range(NCH):
                m_t = mp.tile([P, TB], f16, name=f"m_t{j}")
                nc.gpsimd.dma_start(m_t[:], mask_d[j * P:(j + 1) * P, :])
                m_ts.append(m_t)

            engs = [nc.sync, nc.scalar, nc.gpsimd]

            # ---- Phase A: attn[s, t] = (w1.T @ w2T) * mask on the band ----
            attn_sb = []
            with (
                tc.tile_pool(name="w1p", bufs=4) as w1p,
                tc.tile_pool(name="w2p", bufs=3) as w2p,
                tc.tile_pool(name="psA", bufs=1, space=PSUM) as psA,
            ):
                attn_ps = [
                    psA.tile([P, TB], f32, name=f"attn_ps{j}") for j in range(NCH)
                ]
                for bb in range(DCH // W2PACK):
                    w2_t = w2p.tile([P, W2PACK * TB], f16)
                    nc.scalar.dma_start(w2_t[:], w2T_d[bb])
                    for hb in range(W2PACK // W1PACK):
                        w1_t = w1p.tile([P, W1PACK * SW], f16)
                        nc.sync.dma_start(
                            w1_t[:], w1_d[bb * (W2PACK // W1PACK) + hb])
                        for half in range(W1PACK):
                            k = bb * W2PACK + hb * W1PACK + half
                            w1sl = w1_t[:, half * SW:(half + 1) * SW]
                            w2sl = w2_t[:, (hb * W1PACK + half) * TB:
                                        (hb * W1PACK + half + 1) * TB]
                            for j in range(NCH):
                                lo, hi = _band(j)
                                nc.tensor.matmul(
                                    attn_ps[j][:, lo:hi],
                                    w1sl[:, j * P:(j + 1) * P],
                                    w2sl[:, lo:hi],
                                    start=(k == 0),
                                    stop=(k == DCH - 1),
                                )
                for j in range(NCH):
                    lo, hi = _band(j)
                    a_t = apool.tile([P, TB], f16, name=f"attn_sb{j}")
                    nc.vector.tensor_mul(
                        a_t[:, lo:hi], attn_ps[j][:, lo:hi], m_ts[j][:, lo:hi]
                    )
                    attn_sb.append(a_t)

            # ---- Phase B: yT[t, n] = gelu(attn.T @ xT + b2) on the band ----
            with tc.tile_pool(name="psB", bufs=2, space=PSUM) as psB:
                for g in range(NG):
                    x_t = []
                    for j in range(NCH):
                        xt = xp.tile([P, GN], f16, name="x_t", tag="x_t")
                        nc.gpsimd.dma_start(
                            xt[:], xT_d[j, :, g * GN:(g + 1) * GN]
                        )
                        x_t.append(xt)
                    for q in range(NQ):
                        y_ps = psB.tile([P, GN], f32)
                        for h in range(GN // MMN):
                            nsl = slice(h * MMN, (h + 1) * MMN)
                            for c in range(BANDCH):
                                j = q + c
                                nc.tensor.matmul(
                                    y_ps[:, nsl],
                                    attn_sb[j][:, q * P:(q + 1) * P],
                                    x_t[j][:, nsl],
                                    start=(c == 0),
                                    stop=(c == BANDCH - 1),
                                )
                        y_sb = yp.tile([P, GN], f16)
                        nc.scalar.activation(
                            y_sb[:],
                            y_ps[:],
                            mybir.ActivationFunctionType.Gelu,
                            bias=b2_t[:, q:q + 1],
                            scale=1.0,
                        )
                        st_eng = nc.sync if q % 2 == 0 else nc.scalar
                        st_eng.dma_start(
                            yT_d[q * P:(q + 1) * P, g * GN:(g + 1) * GN], y_sb[:]
                        )

    nc.compile()
    nc.finalize()
    return nc


def _get_nc():
    global _NC
    if _NC is None:
        _NC = _build_module()
    return _NC


def prepare_in_maps(x, w1, w2, b2, sparse_mask):
    x = np.asarray(x, dtype=np.float32)
    w1 = np.asarray(w1, dtype=np.float32)
    w2 = np.asarray(w2, dtype=np.float32)
    b2 = np.asarray(b2, dtype=np.float32)
    sparse_mask = np.asarray(sparse_mask, dtype=np.float32)

    xT = np.ascontiguousarray(x.reshape(N, D).T.astype(np.float16))   # [s, n]
    w2T = np.ascontiguousarray(w2.T.astype(np.float16))               # [d, t]

    # Zero-pad the s axis by MARGIN on both sides so every core's window is
    # a plain slice; mask zeros make the padded rows contribute nothing.
    xT_pad = np.zeros((D + 2 * MARGIN, N), dtype=np.float16)
    xT_pad[MARGIN:MARGIN + D] = xT
    w1_pad = np.zeros((D, D + 2 * MARGIN), dtype=np.float16)
    w1_pad[:, MARGIN:MARGIN + D] = w1.astype(np.float16)
    mask_pad = np.zeros((D + 2 * MARGIN, D), dtype=np.float16)
    mask_pad[MARGIN:MARGIN + D] = sparse_mask.astype(np.float16)

    in_maps = []
    for i in range(NCORES):
        s0 = i * TB           # window start in padded coords
        t0 = i * TB
        w1win = w1_pad[:, s0:s0 + SW]                     # [D, SW]
        # pack W1PACK d-chunks per DMA row: [DCH/W1PACK, P, W1PACK*SW]
        w1_s = (w1win.reshape(DCH // W1PACK, W1PACK, P, SW)
                .transpose(0, 2, 1, 3)
                .reshape(DCH // W1PACK, P, W1PACK * SW))
        w2win = w2T[:, t0:t0 + TB]                        # [D, TB]
        w2_s = (w2win.reshape(DCH // W2PACK, W2PACK, P, TB)
                .transpose(0, 2, 1, 3)
                .reshape(DCH // W2PACK, P, W2PACK * TB))
        in_maps.append({
            "xT_s": np.ascontiguousarray(
                xT_pad[s0:s0 + SW].reshape(NCH, P, N)),
            "w1_s": np.ascontiguousarray(w1_s),
            "w2T_s": np.ascontiguousarray(w2_s),
            "mask_s": np.ascontiguousarray(mask_pad[s0:s0 + SW, t0:t0 + TB]),
            "b2c_s": np.ascontiguousarray(b2[t0:t0 + TB].reshape(NQ, P).T),
        })
    return in_maps


def assemble(results):
    out = np.empty((N, D), dtype=np.float32)
    for i in range(NCORES):
        out[:, i * TB:(i + 1) * TB] = results[i]["yT_s"].T.astype(np.float32)
    return out.reshape(B, T, D)


def kernel(x, w1, w2, b2, sparse_mask):
    from concourse.bass_utils import run_bass_kernel_spmd

    in_maps = prepare_in_maps(x, w1, w2, b2, sparse_mask)
    nc = _get_nc()
    res = run_bass_kernel_spmd(nc, in_maps, list(range(NCORES)))
    return assemble(res.results)
